# revision 1
# baseline (speedup 1.0000x reference)
"""Trainium2 Bass kernel for nn_CollaborativeRNNModel.

Model (per reference):
  per step t (T=100), batch b (B=64), hidden H=128:
    g_u = h @ gate_ku[uid,:,128:256] + gate_bias[128:] + gate_ki[iid,128:]
    u   = sigmoid(g_u)                       (r-half is computed but unused)
    c   = tanh(h @ cand_ku[uid] + cand_bias + cand_ki[iid])
    h'  = u*h + (1-u)*c
  logits = states[B*T, H] @ ws[H, 20001]

Sharding: data-parallel over batch, 8 rows per core.  The per-user weight
matrices for each core's (t, b) sequence are laid out host-side in step
order, so the device streams them with one sequential 256KB DMA per step
instead of 800 row gathers; item-embedding rows are likewise pre-selected
and pre-transposed host-side.  The weight stream is fp8 e3m4 scaled by 64
(the activations rescale by 1/64 and apply the biases via their bias
operand); hidden state, ws, and the logits output are fp16; PSUM stays
fp32.  fp16/fp8 weight loads and matmuls run 4x faster than fp32 on the
PE, and the DMA bytes drop 4x/2x.  The big logits matmul is interleaved
with the recurrence in 512-column PSUM chunks whose SBUF drains alternate
between the vector and scalar engines (gpsimd cannot read PSUM), and the
fp16 logits are upcast to fp32 on the host.  A warmup execution inside
kernel() absorbs a first-run race between the initial input DMAs and the
first compute instructions of a freshly loaded NEFF.
"""

import os

import numpy as np
import ml_dtypes  # noqa: F401  (np "bfloat16"/"float8_*" dtype support)

import concourse.bass as bass  # noqa: F401
import concourse.bacc as bacc
import concourse.tile as tile
import concourse.mybir as mybir
import concourse.bass_utils as bass_utils

H = 128
U = 5000
I = 20000
B = 64
T = 100
N_CORES = 8
BPC = B // N_CORES          # batch rows per core = 8
V = I + 1                   # vocab/items = 20001
NI = BPC * T                # rows per core = 800
VCHUNK = 512                # final-matmul PSUM chunk (one PSUM bank)
STG = 2048                  # output staging width (4 chunks per unit)
PF = 6                      # weight-stream prefetch depth (steps)
F32 = mybir.dt.float32
F16 = mybir.dt.float16
F8 = mybir.dt.float8e3
NP16 = np.float16

# fp8 (e3m4) weight stream: weights scaled by W_SCALE host-side (clipped to
# the e3m4 range); the ki+bias PSUM preload is scaled to match and the
# activations rescale by 1/W_SCALE (exact power of two).
WSEQ_FP8 = os.environ.get("KERNEL_WSEQ_FP8", "1") == "1"
W_SCALE = 64.0
F8MAX = 15.5


def build_nc(t_steps=T):
    """Build and compile the per-core Bass program (SPMD, same on all cores)."""
    ni = BPC * t_steps
    n_mtiles = (ni + 127) // 128

    nc = bacc.Bacc("TRN2", target_bir_lowering=False, debug=False,
                   enable_asserts=False, num_devices=N_CORES)

    WDT = F8 if WSEQ_FP8 else F16

    # DRAM inputs (per core)
    # wseq[t*128 + h, b*256 + k] = [gate_ku[uid,:,128:] | cand_ku[uid]][h, k]
    #   for uid = user_ids[b, t] of this core's batch row b
    wseq_d = nc.dram_tensor("wseq", [t_steps * H, BPC * 2 * H], WDT,
                            kind="ExternalInput")
    # kib[k, t*16 + g*8 + j] = gate_ki[iid(t, b=g*4+j), 128+k]  for j<4
    #                          cand_ki[iid(t, b=g*4+j-4), k]    for j>=4
    kib_d = nc.dram_tensor("kib", [H, 2 * ni], F16, kind="ExternalInput")
    ws_d = nc.dram_tensor("ws", [H, V], F16, kind="ExternalInput")
    h0t_d = nc.dram_tensor("h0t", [H, BPC], F16, kind="ExternalInput")
    bias_u_d = nc.dram_tensor("bias_u", [H, 1], F32, kind="ExternalInput")
    bias_c_d = nc.dram_tensor("bias_c", [H, 1], F32, kind="ExternalInput")
    out_d = nc.dram_tensor("logits", [ni, V], F16, kind="ExternalOutput")

    with tile.TileContext(nc) as tc:
        with (
            tc.tile_pool(name="const", bufs=1) as cpool,
            tc.tile_pool(name="big", bufs=1) as bpool,
            tc.tile_pool(name="w", bufs=PF + 2) as wpool,
            tc.tile_pool(name="sm", bufs=4) as spool,
            tc.tile_pool(name="stage", bufs=4) as stpool,
            tc.tile_pool(name="prec", bufs=3, space="PSUM") as prec,
            tc.tile_pool(name="pfin", bufs=4, space="PSUM") as pfin,
        ):
            # ---- one-time loads ----
            # scalar-engine queue: kib, biases; sync queue: wseq stream
            kib = bpool.tile([128, 2 * ni], F16, tag="kib")
            nc.scalar.dma_start(kib[:], kib_d.ap())
            bias_u = cpool.tile([H, 1], F32, tag="bu")
            nc.scalar.dma_start(bias_u[:], bias_u_d.ap())
            bias_c = cpool.tile([H, 1], F32, tag="bc")
            nc.scalar.dma_start(bias_c[:], bias_c_d.ap())

            # statesT[k, t*8 + b] = state col b BEFORE step t (t=0 -> h0);
            # col 800+ = final states.  fp16 throughout.
            statesT = bpool.tile([H, BPC * (t_steps + 1)], F16, tag="statesT")
            nc.gpsimd.dma_start(statesT[:, 0:BPC], h0t_d.ap())

            # for the fp8 weight stream the ki term is scaled up by W_SCALE
            # to match the scaled matmul; the activations rescale by 1/W_SCALE
            # (the biases are applied unscaled inside the activations)
            if WSEQ_FP8:
                nc.vector.tensor_scalar(out=kib[:], in0=kib[:],
                                        scalar1=W_SCALE, scalar2=None,
                                        op0=mybir.AluOpType.mult)

            # weight-stream prefetch
            wt_tiles = {}

            def issue_wt(t):
                wt = wpool.tile([128, BPC * 2 * H], WDT, tag="wt")
                nc.sync.dma_start(wt[:], wseq_d.ap()[t * H:(t + 1) * H, :])
                wt_tiles[t] = wt

            for t in range(min(PF, t_steps)):
                issue_wt(t)

            # ws resident in SBUF (scalar queue; first needed at ~step 17)
            ws_sb = bpool.tile([H, V], F16, tag="ws")
            nc.scalar.dma_start(ws_sb[:], ws_d.ap())

            # ---- interleaved final matmul machinery ----
            pending = []            # (m, cg) output units, FIFO
            rot = [0]

            def emit_unit():
                m, cg = pending.pop(0)
                lo = m * 128
                mw = min(128, ni - lo)
                gw = min(STG, V - cg)
                lhs = statesT[:, BPC + lo: BPC + lo + mw]
                st = stpool.tile([128, STG], F16, tag="stg")
                for ci in range(cg, cg + gw, VCHUNK):
                    cw = min(VCHUNK, cg + gw - ci)
                    pf = pfin.tile([128, VCHUNK], F32, tag="pf")
                    nc.tensor.matmul(pf[:mw, :cw], lhsT=lhs,
                                     rhs=ws_sb[:, ci:ci + cw],
                                     start=True, stop=True)
                    r = rot[0] % 2
                    rot[0] += 1
                    dst = st[:mw, ci - cg:ci - cg + cw]
                    if r == 0:
                        nc.vector.tensor_copy(dst, pf[:mw, :cw])
                    else:
                        nc.scalar.copy(dst, pf[:mw, :cw])
                nc.gpsimd.dma_start(out_d.ap()[lo:lo + mw, cg:cg + gw],
                                    st[:mw, :gw])

            # ---- recurrence ----
            m_queued = 0
            for t in range(t_steps):
                if t + PF < t_steps:
                    issue_wt(t + PF)
                wt = wt_tiles.pop(t)

                # ps columns: [g0: u(4) c(4) | g1: u(4) c(4)].  The ki terms
                # are preloaded into PSUM and the gate matmuls accumulate on
                # top (start=False), so the activations read PSUM directly.
                ps = prec.tile([128, 2 * BPC], F32, tag="ps")
                nc.vector.tensor_copy(ps[:], kib[:, t * 16:(t + 1) * 16])
                asc = 1.0 / W_SCALE if WSEQ_FP8 else 1.0
                for g in (0, 1):
                    gb = g * 4
                    for j in range(4):
                        b = gb + j
                        hcol = statesT[:, t * BPC + b: t * BPC + b + 1]
                        nc.tensor.matmul(ps[:, g * 8 + j:g * 8 + j + 1],
                                         lhsT=wt[:, b * 256: b * 256 + 128],
                                         rhs=hcol, start=False, stop=True,
                                         skip_group_check=True)
                        nc.tensor.matmul(ps[:, g * 8 + 4 + j:g * 8 + 5 + j],
                                         lhsT=wt[:, b * 256 + 128:(b + 1) * 256],
                                         rhs=hcol, start=False, stop=True,
                                         skip_group_check=True)
                    gc = slice(t * BPC + gb, t * BPC + gb + 4)
                    ngc = slice((t + 1) * BPC + gb, (t + 1) * BPC + gb + 4)
                    uu = spool.tile([128, 4], F16, tag=f"uu{g}")
                    nc.scalar.activation(uu[:], ps[:, g * 8:g * 8 + 4],
                                         mybir.ActivationFunctionType.Sigmoid,
                                         bias=bias_u[:], scale=asc)
                    cc = spool.tile([128, 4], F16, tag=f"cc{g}")
                    nc.scalar.activation(cc[:], ps[:, g * 8 + 4:(g + 1) * 8],
                                         mybir.ActivationFunctionType.Tanh,
                                         bias=bias_c[:], scale=asc)
                    dd = spool.tile([128, 4], F16, tag=f"dd{g}")
                    nc.vector.tensor_sub(dd[:], statesT[:, gc], cc[:])
                    ee = spool.tile([128, 4], F16, tag=f"ee{g}")
                    nc.vector.tensor_mul(ee[:], uu[:], dd[:])
                    nc.vector.tensor_add(statesT[:, ngc], cc[:], ee[:])
                    # smooth the drain: 1 unit/step mid-kernel (spreads PE
                    # filler into every h'-wait window), 2/step near the end
                    if pending and (g == 1 or t >= 90):
                        emit_unit()

                while m_queued < n_mtiles and (m_queued + 1) * 128 <= (t + 1) * BPC:
                    for cg in range(0, V, STG):
                        pending.append((m_queued, cg))
                    m_queued += 1

            while m_queued < n_mtiles:
                for cg in range(0, V, STG):
                    pending.append((m_queued, cg))
                m_queued += 1
            while pending:
                emit_unit()

    nc.compile()
    return nc


def prep_inputs(user_ids, item_ids, h0, gate_ku, gate_ki, gate_bias,
                cand_ku, cand_ki, cand_bias, ws, t_steps=T):
    """Host-side sharding/layout -> per-core in_maps."""
    ni = BPC * t_steps
    # combined per-user table [U+1, 128, 256] = [gate_u_half | cand]
    if WSEQ_FP8:
        f8 = ml_dtypes.float8_e3m4
        wu16 = np.clip(np.ascontiguousarray(gate_ku[:, :, H:]) * W_SCALE,
                       -F8MAX, F8MAX).astype(f8)
        wc16 = np.clip(np.ascontiguousarray(cand_ku) * W_SCALE,
                       -F8MAX, F8MAX).astype(f8)
    else:
        wu16 = np.ascontiguousarray(gate_ku[:, :, H:]).astype(NP16)
        wc16 = np.ascontiguousarray(cand_ku).astype(NP16)
    ws16 = np.ascontiguousarray(ws).astype(NP16)
    bias_u = np.ascontiguousarray(gate_bias[H:].reshape(H, 1), np.float32)
    bias_c = np.ascontiguousarray(cand_bias.reshape(H, 1), np.float32)
    gki16 = np.ascontiguousarray(gate_ki[:, H:]).astype(NP16)   # [V, 128]
    cki16 = np.ascontiguousarray(cand_ki).astype(NP16)          # [V, 128]

    in_maps = []
    for c in range(N_CORES):
        rows = slice(c * BPC, (c + 1) * BPC)
        uid_t = np.asarray(user_ids[rows, :t_steps], np.int64).T    # [T, 8]
        iid_flat = np.asarray(item_ids[rows, :t_steps], np.int64).T.reshape(-1)
        # weight stream: [T,8,128,128]x2 -> [T,128,8,256] -> [T*128, 2048]
        wseq = np.concatenate([wu16[uid_t], wc16[uid_t]], axis=3)
        wseq = np.ascontiguousarray(wseq.transpose(0, 2, 1, 3)).reshape(
            t_steps * H, BPC * 2 * H)
        # ki table, interleaved per step: col t*16+g*8+j = u-ki of b=g*4+j
        # (j<4) / cand-ki of b=g*4+j-4 (j>=4), transposed to [128, 16T]
        kib_u = gki16[iid_flat].reshape(t_steps, 2, 4, H)
        kib_c = cki16[iid_flat].reshape(t_steps, 2, 4, H)
        kib = np.ascontiguousarray(
            np.concatenate([kib_u, kib_c], axis=2).reshape(t_steps * 16, H).T)
        h0t = np.ascontiguousarray(h0[rows].T.astype(NP16))
        in_maps.append({
            "wseq": wseq, "kib": kib, "ws": ws16,
            "h0t": h0t, "bias_u": bias_u, "bias_c": bias_c,
        })
    return in_maps


def assemble_output(results, t_steps=T):
    ni = BPC * t_steps
    out = np.empty((B * t_steps, V), np.float32)
    for c in range(N_CORES):
        blk = results[c]["logits"]  # [ni, V] fp16, rows i = t*8+b
        out[c * ni:(c + 1) * ni] = (
            blk.reshape(t_steps, BPC, V).transpose(1, 0, 2)
            .reshape(ni, V).astype(np.float32))
    return out


_NC_CACHE = {}


def _get_nc(t_steps=T):
    key = (t_steps, WSEQ_FP8)
    if key not in _NC_CACHE:
        _NC_CACHE[key] = build_nc(t_steps)
    return _NC_CACHE[key]


_WARMED = set()


def kernel(user_ids, item_ids, h0, gate_ku, gate_ki, gate_bias,
           cand_ku, cand_ki, cand_bias, ws, trace=False):
    nc = _get_nc(T)
    in_maps = prep_inputs(np.asarray(user_ids), np.asarray(item_ids),
                          np.asarray(h0), np.asarray(gate_ku),
                          np.asarray(gate_ki), np.asarray(gate_bias),
                          np.asarray(cand_ku), np.asarray(cand_ki),
                          np.asarray(cand_bias), np.asarray(ws))
    # The very first execution of a freshly-loaded NEFF can race its initial
    # input DMAs against the first compute instructions (observed: step-0
    # state garbage that decays like u^t).  Run once untraced to warm the
    # device, then run for real.
    if id(nc) not in _WARMED:
        bass_utils.run_bass_kernel_spmd(
            nc, in_maps, core_ids=list(range(N_CORES)), trace=False)
        _WARMED.add(id(nc))
    res = bass_utils.run_bass_kernel_spmd(
        nc, in_maps, core_ids=list(range(N_CORES)), trace=trace)
    out = assemble_output(res.results)
    if trace:
        kernel.last_result = res
    return out



# revision 3
# speedup vs baseline: 1.1658x; 1.1658x over previous
"""Trainium2 Bass kernel for nn_CollaborativeRNNModel.

Model (per reference):
  per step t (T=100), batch b (B=64), hidden H=128:
    u   = sigmoid(h @ gate_ku[uid,:,128:] + gate_bias[128:] + gate_ki[iid,128:])
    c   = tanh(h @ cand_ku[uid] + cand_bias + cand_ki[iid])
    h'  = u*h + (1-u)*c
  logits = states[B*T, H] @ ws[H, 20001]

Sharding: data-parallel over batch, 8 rows per core.  Per-user weights are
pre-gathered host-side into a fp8(e3m4) stream (one 512KB DMA per 2 steps).

Key device-side structure:
  - State is stored as S = 2h (fp16).  Both gate matmuls use rhs = S with
    host-side weight scales gamma_u=32, gamma_c=64, so PSUM holds 64*x_u and
    128*x_c.  A SINGLE tanh activation per step (scale 1/128) then yields
    t_u = tanh(x_u/2) (so u = (1+t_u)/2) and t_c = tanh(x_c) = c; the biases
    and item embeddings are pre-folded into a fp32 PSUM preload.  The state
    update S' = S/2 + t_c + t_u*(S/2 - t_c) takes 4 fused DVE ops.
  - The logits matmul (lhsT = S columns, rhs = 32*ws in fp8) is interleaved
    with the recurrence in 1024-col PSUM chunks; drains (scale 1/64, fp32
    PSUM -> fp16 SBUF) alternate between the scalar and vector engines and
    2048-col staging tiles stream out via the gpsimd DMA queue (tail via all
    three queues).
  - A warmup execution inside kernel() absorbs a first-run race between the
    initial input DMAs and the first compute instructions of a fresh NEFF.
"""

import numpy as np
import ml_dtypes  # noqa: F401  (np fp8 dtype support)

import concourse.bass as bass  # noqa: F401
import concourse.bacc as bacc
import concourse.tile as tile
import concourse.mybir as mybir
import concourse.bass_utils as bass_utils

H = 128
U = 5000
I = 20000
B = 64
T = 100
N_CORES = 8
BPC = B // N_CORES          # batch rows per core = 8
V = I + 1                   # vocab/items = 20001
NI = BPC * T                # rows per core = 800
CHUNK = 1024                # logits PSUM chunk (2 banks)
STG = 2048                  # output staging width (2 chunks per unit)
PFW = 3                     # wseq prefetch depth (2-step chunks)
F32 = mybir.dt.float32
F16 = mybir.dt.float16
F8 = mybir.dt.float8e3
NP16 = np.float16

F8MAX = 15.5
G_U = 32.0                  # gate (u-half) weight scale
G_C = 64.0                  # cand weight scale
WS_SCALE = 32.0             # ws weight scale
ACT_SCALE = 1.0 / 128.0     # tanh input scale
DRAIN_SCALE = 1.0 / 64.0    # logits drain scale (2h * 32ws = 64*logits)


def build_nc(t_steps=T):
    """Build and compile the per-core Bass program (SPMD, same on all cores)."""
    ni = BPC * t_steps
    n_mtiles = (ni + 127) // 128
    n_wchunks = t_steps // 2

    nc = bacc.Bacc("TRN2", target_bir_lowering=False, debug=False,
                   enable_asserts=False, num_devices=N_CORES)

    # DRAM inputs (per core)
    # wseq[s*128 + k, st*2048 + b*256 + j] for step t = 2s+st:
    #   j<128: 32*gate_ku[uid(t,b), k, 128+j];  j>=128: 64*cand_ku[uid(t,b), k, j-128]
    wseq_d = nc.dram_tensor("wseq", [n_wchunks * H, 2 * BPC * 2 * H], F8,
                            kind="ExternalInput")
    # kib[k, t*16 + j] = 64*(gate_ki[iid(t,b=j),128+k] + gate_bias[128+k])   (j<8)
    #                    128*(cand_ki[iid(t,b=j-8),k] + cand_bias[k])        (j>=8)
    kib_d = nc.dram_tensor("kib", [H, 16 * t_steps], F32, kind="ExternalInput")
    ws_d = nc.dram_tensor("ws", [H, V], F8, kind="ExternalInput")
    s0_d = nc.dram_tensor("s0", [H, BPC], F16, kind="ExternalInput")
    out_d = nc.dram_tensor("logits", [ni, V], F16, kind="ExternalOutput")

    with tile.TileContext(nc) as tc:
        with (
            tc.tile_pool(name="big", bufs=1) as bpool,
            tc.tile_pool(name="w", bufs=PFW + 2) as wpool,
            tc.tile_pool(name="sm", bufs=4) as spool,
            tc.tile_pool(name="stage", bufs=4) as stpool,
            tc.tile_pool(name="prec", bufs=2, space="PSUM") as prec,
            tc.tile_pool(name="pfin", bufs=3, space="PSUM") as pfin,
        ):
            # ---- one-time loads ----
            kib = bpool.tile([128, 16 * t_steps], F32, tag="kib")
            nc.scalar.dma_start(kib[:], kib_d.ap())

            # statesT[k, t*8 + b] = S col b BEFORE step t (t=0 -> 2*h0)
            statesT = bpool.tile([H, BPC * (t_steps + 1)], F16, tag="statesT")
            nc.scalar.dma_start(statesT[:, 0:BPC], s0_d.ap())

            # weight-stream prefetch (2 steps per chunk)
            wt_tiles = {}

            def issue_wt(s):
                wt = wpool.tile([128, 2 * BPC * 2 * H], F8, tag="wt")
                nc.sync.dma_start(wt[:], wseq_d.ap()[s * H:(s + 1) * H, :])
                wt_tiles[s] = wt

            for s in range(min(PFW, n_wchunks)):
                issue_wt(s)

            # ws resident in SBUF (first needed at ~step 17)
            ws_sb = bpool.tile([H, V], F8, tag="ws")
            nc.scalar.dma_start(ws_sb[:], ws_d.ap())

            # ---- interleaved logits-matmul machinery ----
            # chunk queue: (m, cg, w, unit_key); units are STG-wide staging
            # tiles flushed to DRAM when both chunk halves are drained.
            pending = []
            unit_state = {}     # unit_key -> [stage_tile, n_remaining, m, cg, gw]
            m_queued = [0]
            dma_rot = [0]

            def queue_mtile():
                m = m_queued[0]
                lo = m * 128
                mw = min(128, ni - lo)
                for cg in range(0, V, STG):
                    gw = min(STG, V - cg)
                    key = (m, cg)
                    unit_state[key] = [None, 0, lo, mw, cg, gw]
                    for ci in range(cg, cg + gw, CHUNK):
                        cw = min(CHUNK, cg + gw - ci)
                        pending.append((ci, cw, key))
                        unit_state[key][1] += 1
                m_queued[0] += 1

            def emit_chunk(use_scalar, tail=False):
                if not pending:
                    return
                ci, cw, key = pending.pop(0)
                st = unit_state[key]
                _, _, lo, mw, cg, gw = st
                if st[0] is None:
                    st[0] = stpool.tile([128, STG], F16, tag="stg",
                                        name="stg")
                stg = st[0]
                lhs = statesT[:, BPC + lo: BPC + lo + mw]
                pf = pfin.tile([128, CHUNK], F32, tag="pf")
                for q in range(0, cw, 512):
                    qw = min(512, cw - q)
                    nc.tensor.matmul(pf[:mw, q:q + qw], lhsT=lhs,
                                     rhs=ws_sb[:, ci + q:ci + q + qw],
                                     start=True, stop=True,
                                     skip_group_check=True)
                dst = stg[:mw, ci - cg:ci - cg + cw]
                if use_scalar:
                    nc.scalar.activation(dst, pf[:mw, :cw],
                                         mybir.ActivationFunctionType.Copy,
                                         bias=0.0, scale=DRAIN_SCALE)
                else:
                    nc.vector.tensor_scalar(out=dst, in0=pf[:mw, :cw],
                                            scalar1=DRAIN_SCALE, scalar2=None,
                                            op0=mybir.AluOpType.mult)
                st[1] -= 1
                if st[1] == 0:
                    # unit complete -> stream out
                    if tail:
                        eng = (nc.gpsimd, nc.scalar, nc.sync)[dma_rot[0] % 3]
                        dma_rot[0] += 1
                    else:
                        eng = nc.gpsimd
                    eng.dma_start(out_d.ap()[lo:lo + mw, cg:cg + gw],
                                  stg[:mw, :gw])
                    del unit_state[key]

            # ---- recurrence ----
            for t in range(t_steps):
                if t % 2 == 0:
                    s = t // 2
                    if s + PFW < n_wchunks:
                        issue_wt(s + PFW)
                wt = wt_tiles[t // 2]
                if t % 2 == 1:
                    del wt_tiles[t // 2]
                wbase = (t % 2) * (BPC * 2 * H)

                # PSUM preload with folded ki+bias terms; gate matmuls
                # accumulate on top (start=False).
                ps = prec.tile([128, 2 * BPC], F32, tag="ps")
                nc.vector.tensor_copy(ps[:], kib[:, t * 16:(t + 1) * 16])

                for b in range(BPC):
                    scol = statesT[:, t * BPC + b: t * BPC + b + 1]
                    w0 = wbase + b * 256
                    nc.tensor.matmul(ps[:, b:b + 1],
                                     lhsT=wt[:, w0:w0 + 128],
                                     rhs=scol, start=False, stop=True,
                                     skip_group_check=True)
                    nc.tensor.matmul(ps[:, BPC + b:BPC + b + 1],
                                     lhsT=wt[:, w0 + 128:w0 + 256],
                                     rhs=scol, start=False, stop=True,
                                     skip_group_check=True)

                # one tanh for both halves: t_u = tanh(x_u/2), t_c = tanh(x_c)
                tt = spool.tile([128, 2 * BPC], F16, tag="tt")
                nc.scalar.activation(tt[:], ps[:],
                                     mybir.ActivationFunctionType.Tanh,
                                     scale=ACT_SCALE)

                # S' = (S/2 + t_c) + t_u*(S/2 - t_c)
                scur = statesT[:, t * BPC:(t + 1) * BPC]
                t_u = tt[:, 0:BPC]
                t_c = tt[:, BPC:2 * BPC]
                t0 = spool.tile([128, BPC], F16, tag="t0")
                nc.vector.scalar_tensor_tensor(
                    out=t0[:], in0=scur, scalar=0.5, in1=t_c,
                    op0=mybir.AluOpType.mult, op1=mybir.AluOpType.add)
                t1 = spool.tile([128, BPC], F16, tag="t1")
                nc.vector.scalar_tensor_tensor(
                    out=t1[:], in0=scur, scalar=0.5, in1=t_c,
                    op0=mybir.AluOpType.mult, op1=mybir.AluOpType.subtract)
                t2 = spool.tile([128, BPC], F16, tag="t2")
                nc.vector.tensor_mul(t2[:], t_u, t1[:])
                nc.vector.tensor_add(statesT[:, (t + 1) * BPC:(t + 2) * BPC],
                                     t0[:], t2[:])

                # queue newly-completed m-tiles, then drain chunks
                while m_queued[0] < n_mtiles and \
                        (m_queued[0] + 1) * 128 <= (t + 1) * BPC:
                    queue_mtile()
                if t >= 90:
                    emit_chunk(use_scalar=True)
                    emit_chunk(use_scalar=False)
                else:
                    turn = t % 3
                    if turn == 0:
                        emit_chunk(use_scalar=True)
                    elif turn == 1:
                        emit_chunk(use_scalar=False)
                    else:
                        emit_chunk(use_scalar=True)
                        emit_chunk(use_scalar=False)

            # ---- tail ----
            while m_queued[0] < n_mtiles:
                queue_mtile()
            use_scalar = True
            while pending:
                emit_chunk(use_scalar=use_scalar, tail=True)
                use_scalar = not use_scalar

    nc.compile()
    return nc


def prep_inputs(user_ids, item_ids, h0, gate_ku, gate_ki, gate_bias,
                cand_ku, cand_ki, cand_bias, ws, t_steps=T):
    """Host-side sharding/layout -> per-core in_maps."""
    f8 = ml_dtypes.float8_e3m4
    wu8 = np.clip(np.ascontiguousarray(gate_ku[:, :, H:]) * G_U,
                  -F8MAX, F8MAX).astype(f8)
    wc8 = np.clip(np.ascontiguousarray(cand_ku) * G_C,
                  -F8MAX, F8MAX).astype(f8)
    ws8 = np.clip(np.ascontiguousarray(ws) * WS_SCALE,
                  -F8MAX, F8MAX).astype(f8)
    # folded ki + bias tables (fp32)
    gki = (np.asarray(gate_ki[:, H:], np.float32)
           + np.asarray(gate_bias[H:], np.float32)) * 64.0      # [V, 128]
    cki = (np.asarray(cand_ki, np.float32)
           + np.asarray(cand_bias, np.float32)) * 128.0          # [V, 128]

    in_maps = []
    for c in range(N_CORES):
        rows = slice(c * BPC, (c + 1) * BPC)
        uid_t = np.asarray(user_ids[rows, :t_steps], np.int64).T    # [T, 8]
        iid_t = np.asarray(item_ids[rows, :t_steps], np.int64).T    # [T, 8]
        # weight stream: [T,8,128,256] -> [T,128,8,256] -> 2-step chunks
        wseq = np.concatenate([wu8[uid_t], wc8[uid_t]], axis=3)
        wseq = np.ascontiguousarray(wseq.transpose(0, 2, 1, 3)).reshape(
            t_steps * H, BPC * 2 * H)
        wseq = np.ascontiguousarray(
            wseq.reshape(t_steps // 2, 2, H, BPC * 2 * H)
            .transpose(0, 2, 1, 3)).reshape(t_steps // 2 * H, 2 * BPC * 2 * H)
        # kib columns per step: [u(8) | c(8)], transposed to [128, 16T]
        kib = np.concatenate([gki[iid_t], cki[iid_t]], axis=1)   # [T,16,128]
        kib = np.ascontiguousarray(kib.reshape(t_steps * 16, H).T,
                                   np.float32)
        s0 = np.ascontiguousarray((2.0 * np.asarray(h0[rows], np.float32)).T
                                  .astype(NP16))
        in_maps.append({"wseq": wseq, "kib": kib, "ws": ws8, "s0": s0})
    return in_maps


def assemble_output(results, t_steps=T):
    ni = BPC * t_steps
    out = np.empty((B * t_steps, V), np.float32)
    for c in range(N_CORES):
        blk = results[c]["logits"]  # [ni, V] fp16, rows i = t*8+b
        out[c * ni:(c + 1) * ni] = (
            blk.reshape(t_steps, BPC, V).transpose(1, 0, 2)
            .reshape(ni, V).astype(np.float32))
    return out


_NC_CACHE = {}


def _get_nc(t_steps=T):
    if t_steps not in _NC_CACHE:
        _NC_CACHE[t_steps] = build_nc(t_steps)
    return _NC_CACHE[t_steps]


_WARMED = set()


def kernel(user_ids, item_ids, h0, gate_ku, gate_ki, gate_bias,
           cand_ku, cand_ki, cand_bias, ws, trace=False):
    nc = _get_nc(T)
    in_maps = prep_inputs(np.asarray(user_ids), np.asarray(item_ids),
                          np.asarray(h0), np.asarray(gate_ku),
                          np.asarray(gate_ki), np.asarray(gate_bias),
                          np.asarray(cand_ku), np.asarray(cand_ki),
                          np.asarray(cand_bias), np.asarray(ws))
    # First execution of a fresh NEFF can race its initial input DMAs against
    # the first compute instructions; run once untraced to warm the device.
    if id(nc) not in _WARMED:
        bass_utils.run_bass_kernel_spmd(
            nc, in_maps, core_ids=list(range(N_CORES)), trace=False)
        _WARMED.add(id(nc))
    res = bass_utils.run_bass_kernel_spmd(
        nc, in_maps, core_ids=list(range(N_CORES)), trace=trace)
    out = assemble_output(res.results)
    if trace:
        kernel.last_result = res
    return out


# revision 8
# speedup vs baseline: 1.2640x; 1.0843x over previous
"""Trainium2 Bass kernel for nn_CollaborativeRNNModel.

Model (per reference):
  per step t (T=100), batch b (B=64), hidden H=128:
    u   = sigmoid(h @ gate_ku[uid,:,128:] + gate_bias[128:] + gate_ki[iid,128:])
    c   = tanh(h @ cand_ku[uid] + cand_bias + cand_ki[iid])
    h'  = u*h + (1-u)*c
  logits = states[B*T, H] @ ws[H, 20001]

Sharding: data-parallel over batch, 8 rows per core.  Per-user weights are
pre-gathered host-side into a fp8(e3m4) stream (one 512KB DMA per 2 steps).

Device-side structure:
  - State is stored as S = 2h (fp16).  Both gate matmuls use rhs = S with
    host-side weight scales gamma_u=32, gamma_c=64, so PSUM holds 64*x_u and
    128*x_c.  A SINGLE tanh activation per step (scale 1/128) yields
    t_u = tanh(x_u/2) (u = (1+t_u)/2) and t_c = tanh(x_c) = c; biases and
    item embeddings are pre-folded into a fp32 PSUM preload (the preload for
    step t+1 is issued before the t update so it runs in the matmul window).
  - The state update S' = (S/2 + t_c) + t_u*(S/2 - t_c) takes 4 fused DVE
    ops; per-step chain is MMs -> tanh -> 4 DVE ops -> next MMs.
  - The logits matmul (lhsT = S columns, rhs = fp16 ws; logits are tiny so
    fp8 ws would alone cost ~1.3e-2 rel err) is interleaved in 512-col PSUM
    chunks whose matmuls are issued one step BEFORE their drains, so drains
    (scale 1/64, fp32 PSUM -> fp16 SBUF; 1 on vector + up to 2 on scalar per
    step) never stall an engine FIFO waiting on the PE.  2048-col staging
    tiles stream out on the gpsimd queue (tail across all three queues).
  - A warmup execution inside kernel() absorbs a first-run race between the
    initial input DMAs and the first compute instructions of a fresh NEFF.
"""

import numpy as np
import ml_dtypes  # noqa: F401  (np fp8 dtype support)

import concourse.bass as bass  # noqa: F401
import concourse.bacc as bacc
import concourse.tile as tile
import concourse.mybir as mybir
import concourse.bass_utils as bass_utils

H = 128
U = 5000
I = 20000
B = 64
T = 100
N_CORES = 8
BPC = B // N_CORES          # batch rows per core = 8
V = I + 1                   # vocab/items = 20001
NI = BPC * T                # rows per core = 800
CHUNK = 1024                # logits PSUM chunk (two banks)
STG = 2048                  # output staging width (2 chunks per unit)
PFW = 3                     # wseq prefetch depth (2-step chunks)
PFIN_BUFS = 3               # in-flight logits PSUM chunks
F32 = mybir.dt.float32
F16 = mybir.dt.float16
F8 = mybir.dt.float8e3
NP16 = np.float16

F8MAX = 15.5
G_U = 32.0                  # gate (u-half) weight scale
G_C = 64.0                  # cand weight scale
ACT_SCALE = 1.0 / 128.0     # tanh input scale
DRAIN_SCALE = 0.5           # logits drain scale (PSUM = S @ ws = 2*logits)


def build_nc(t_steps=T):
    """Build and compile the per-core Bass program (SPMD, same on all cores)."""
    ni = BPC * t_steps
    n_mtiles = (ni + 127) // 128
    n_wchunks = t_steps // 2

    nc = bacc.Bacc("TRN2", target_bir_lowering=False, debug=False,
                   enable_asserts=False, num_devices=N_CORES)

    # DRAM inputs (per core)
    # wseq[s*128 + k, st*2048 + b*256 + j] for step t = 2s+st:
    #   j<128: 32*gate_ku[uid(t,b), k, 128+j];  j>=128: 64*cand_ku[uid(t,b), k, j-128]
    wseq_d = nc.dram_tensor("wseq", [n_wchunks * H, 2 * BPC * 2 * H], F8,
                            kind="ExternalInput")
    # kib[k, t*16 + j] = 64*(gate_ki[iid(t,b=j),128+k] + gate_bias[128+k])   (j<8)
    #                    128*(cand_ki[iid(t,b=j-8),k] + cand_bias[k])        (j>=8)
    kib_d = nc.dram_tensor("kib", [H, 16 * t_steps], F32, kind="ExternalInput")
    ws_d = nc.dram_tensor("ws", [H, V], F16, kind="ExternalInput")
    s0_d = nc.dram_tensor("s0", [H, BPC], F16, kind="ExternalInput")
    out_d = nc.dram_tensor("logits", [ni, V], F16, kind="ExternalOutput")

    with tile.TileContext(nc) as tc:
        with (
            tc.tile_pool(name="big", bufs=1) as bpool,
            tc.tile_pool(name="w", bufs=PFW + 2) as wpool,
            tc.tile_pool(name="sm", bufs=4) as spool,
            tc.tile_pool(name="stage", bufs=4) as stpool,
            tc.tile_pool(name="prec", bufs=2, space="PSUM") as prec,
            tc.tile_pool(name="pfin", bufs=PFIN_BUFS, space="PSUM") as pfin,
        ):
            # ---- one-time loads ----
            kib = bpool.tile([128, 16 * t_steps], F32, tag="kib")
            nc.scalar.dma_start(kib[:], kib_d.ap())

            # statesT[k, t*8 + b] = S col b BEFORE step t (t=0 -> 2*h0)
            statesT = bpool.tile([H, BPC * (t_steps + 1)], F16, tag="statesT")
            nc.scalar.dma_start(statesT[:, 0:BPC], s0_d.ap())

            # weight-stream prefetch (2 steps per chunk)
            wt_tiles = {}

            def issue_wt(s):
                wt = wpool.tile([128, 2 * BPC * 2 * H], F8, tag="wt")
                nc.sync.dma_start(wt[:], wseq_d.ap()[s * H:(s + 1) * H, :])
                wt_tiles[s] = wt

            for s in range(min(PFW, n_wchunks)):
                issue_wt(s)

            # ws resident in SBUF (first needed at ~step 17)
            ws_sb = bpool.tile([H, V], F16, tag="ws")
            nc.scalar.dma_start(ws_sb[:], ws_d.ap())

            # ---- interleaved logits-matmul machinery ----
            # chunk lifecycle: mm_pending -> (chunk MMs issued, pfin tile
            # in flight) -> drain_ready -> (drained into stage) -> unit DMA
            mm_pending = []     # (ci, cw, unit_key)
            drain_ready = []    # (pf_tile, ci, cw, unit_key)
            unit_state = {}     # key -> [stage, nrem, lo, mw, cg, gw]
            m_queued = [0]
            dma_rot = [0]

            def queue_mtile():
                m = m_queued[0]
                lo = m * 128
                mw = min(128, ni - lo)
                for cg in range(0, V, STG):
                    gw = min(STG, V - cg)
                    key = (m, cg)
                    unit_state[key] = [None, 0, lo, mw, cg, gw]
                    for ci in range(cg, cg + gw, CHUNK):
                        cw = min(CHUNK, cg + gw - ci)
                        mm_pending.append((ci, cw, key))
                        unit_state[key][1] += 1
                m_queued[0] += 1

            def issue_chunk_mms(maxn):
                # leave one pfin buffer as slack for drains still executing,
                # so a chunk matmul never blocks the PE queue on buffer reuse
                n = min(maxn, PFIN_BUFS - 1 - len(drain_ready),
                        len(mm_pending))
                for _ in range(max(0, n)):
                    ci, cw, key = mm_pending.pop(0)
                    lo, mw = unit_state[key][2], unit_state[key][3]
                    lhs = statesT[:, BPC + lo: BPC + lo + mw]
                    pf = pfin.tile([128, CHUNK], F32, tag="pf", name="pf")
                    for q in range(0, cw, 512):
                        qw = min(512, cw - q)
                        nc.tensor.matmul(pf[:mw, q:q + qw], lhsT=lhs,
                                         rhs=ws_sb[:, ci + q:ci + q + qw],
                                         start=True, stop=True,
                                         skip_group_check=True)
                    drain_ready.append((pf, ci, cw, key))

            def drain_chunk(use_scalar, tail=False):
                if not drain_ready:
                    return
                pf, ci, cw, key = drain_ready.pop(0)
                st = unit_state[key]
                _, _, lo, mw, cg, gw = st
                if st[0] is None:
                    st[0] = stpool.tile([128, STG], F16, tag="stg",
                                        name="stg")
                stg = st[0]
                dst = stg[:mw, ci - cg:ci - cg + cw]
                if use_scalar:
                    nc.scalar.activation(dst, pf[:mw, :cw],
                                         mybir.ActivationFunctionType.Copy,
                                         bias=0.0, scale=DRAIN_SCALE)
                else:
                    nc.vector.tensor_scalar(out=dst, in0=pf[:mw, :cw],
                                            scalar1=DRAIN_SCALE, scalar2=None,
                                            op0=mybir.AluOpType.mult)
                st[1] -= 1
                if st[1] == 0:
                    if tail:
                        eng = (nc.gpsimd, nc.scalar, nc.sync)[dma_rot[0] % 3]
                        dma_rot[0] += 1
                    else:
                        eng = nc.gpsimd
                    eng.dma_start(out_d.ap()[lo:lo + mw, cg:cg + gw],
                                  stg[:mw, :gw])
                    del unit_state[key]

            # ---- recurrence ----
            ps_tiles = {}
            ps_tiles[0] = prec.tile([128, 2 * BPC], F32, tag="ps", name="ps")
            nc.vector.tensor_copy(ps_tiles[0][:], kib[:, 0:16])

            for t in range(t_steps):
                if t % 2 == 0:
                    s = t // 2
                    if s + PFW < n_wchunks:
                        issue_wt(s + PFW)
                wt = wt_tiles[t // 2]
                if t % 2 == 1:
                    del wt_tiles[t // 2]
                wbase = (t % 2) * (BPC * 2 * H)
                ps = ps_tiles.pop(t)

                for b in range(BPC):
                    scol = statesT[:, t * BPC + b: t * BPC + b + 1]
                    w0 = wbase + b * 256
                    nc.tensor.matmul(ps[:, b:b + 1],
                                     lhsT=wt[:, w0:w0 + 128],
                                     rhs=scol, start=False, stop=True,
                                     skip_group_check=True)
                    nc.tensor.matmul(ps[:, BPC + b:BPC + b + 1],
                                     lhsT=wt[:, w0 + 128:w0 + 256],
                                     rhs=scol, start=False, stop=True,
                                     skip_group_check=True)

                # logits chunk MMs ride the PE queue right behind the small
                # MMs; their drains happen NEXT step so they never stall.
                while m_queued[0] < n_mtiles and \
                        (m_queued[0] + 1) * 128 <= (t + 1) * BPC:
                    queue_mtile()
                issue_chunk_mms(3)

                # preload for t+1 BEFORE the update ops: runs on DVE during
                # the matmul/tanh window, off the critical path.
                if t + 1 < t_steps:
                    psn = prec.tile([128, 2 * BPC], F32, tag="ps", name="ps")
                    nc.vector.tensor_copy(psn[:],
                                          kib[:, (t + 1) * 16:(t + 2) * 16])
                    ps_tiles[t + 1] = psn

                # one tanh for both halves: t_u = tanh(x_u/2), t_c = tanh(x_c)
                tt = spool.tile([128, 2 * BPC], F16, tag="tt")
                nc.scalar.activation(tt[:], ps[:],
                                     mybir.ActivationFunctionType.Tanh,
                                     scale=ACT_SCALE)

                # S' = (S/2 + t_c) + t_u*(S/2 - t_c)
                scur = statesT[:, t * BPC:(t + 1) * BPC]
                t_u = tt[:, 0:BPC]
                t_c = tt[:, BPC:2 * BPC]
                t0 = spool.tile([128, BPC], F16, tag="t0")
                nc.vector.scalar_tensor_tensor(
                    out=t0[:], in0=scur, scalar=0.5, in1=t_c,
                    op0=mybir.AluOpType.mult, op1=mybir.AluOpType.add)
                t1 = spool.tile([128, BPC], F16, tag="t1")
                nc.vector.scalar_tensor_tensor(
                    out=t1[:], in0=scur, scalar=0.5, in1=t_c,
                    op0=mybir.AluOpType.mult, op1=mybir.AluOpType.subtract)
                t2 = spool.tile([128, BPC], F16, tag="t2")
                nc.vector.tensor_mul(t2[:], t_u, t1[:])
                nc.vector.tensor_add(statesT[:, (t + 1) * BPC:(t + 2) * BPC],
                                     t0[:], t2[:])

                # drains for chunks whose MMs were issued in prior steps
                drain_chunk(use_scalar=True)
                if t % 3 == 2 or t >= 90:
                    drain_chunk(use_scalar=False)

            # ---- tail ----
            while m_queued[0] < n_mtiles:
                queue_mtile()
            use_scalar = True
            while mm_pending or drain_ready:
                issue_chunk_mms(2)
                drain_chunk(use_scalar=use_scalar, tail=True)
                use_scalar = not use_scalar

    nc.compile()
    return nc


def prep_inputs(user_ids, item_ids, h0, gate_ku, gate_ki, gate_bias,
                cand_ku, cand_ki, cand_bias, ws, t_steps=T):
    """Host-side sharding/layout -> per-core in_maps."""
    f8 = ml_dtypes.float8_e3m4
    wu8 = np.clip(np.ascontiguousarray(gate_ku[:, :, H:]) * G_U,
                  -F8MAX, F8MAX).astype(f8)
    wc8 = np.clip(np.ascontiguousarray(cand_ku) * G_C,
                  -F8MAX, F8MAX).astype(f8)
    ws16 = np.ascontiguousarray(ws).astype(NP16)
    # folded ki + bias tables (fp32)
    gki = (np.asarray(gate_ki[:, H:], np.float32)
           + np.asarray(gate_bias[H:], np.float32)) * 64.0      # [V, 128]
    cki = (np.asarray(cand_ki, np.float32)
           + np.asarray(cand_bias, np.float32)) * 128.0          # [V, 128]

    in_maps = []
    for c in range(N_CORES):
        rows = slice(c * BPC, (c + 1) * BPC)
        uid_t = np.asarray(user_ids[rows, :t_steps], np.int64).T    # [T, 8]
        iid_t = np.asarray(item_ids[rows, :t_steps], np.int64).T    # [T, 8]
        # weight stream: [T,8,128,256] -> [T,128,8,256] -> 2-step chunks
        wseq = np.concatenate([wu8[uid_t], wc8[uid_t]], axis=3)
        wseq = np.ascontiguousarray(wseq.transpose(0, 2, 1, 3)).reshape(
            t_steps * H, BPC * 2 * H)
        wseq = np.ascontiguousarray(
            wseq.reshape(t_steps // 2, 2, H, BPC * 2 * H)
            .transpose(0, 2, 1, 3)).reshape(t_steps // 2 * H, 2 * BPC * 2 * H)
        # kib columns per step: [u(8) | c(8)], transposed to [128, 16T]
        kib = np.concatenate([gki[iid_t], cki[iid_t]], axis=1)   # [T,16,128]
        kib = np.ascontiguousarray(kib.reshape(t_steps * 16, H).T,
                                   np.float32)
        s0 = np.ascontiguousarray((2.0 * np.asarray(h0[rows], np.float32)).T
                                  .astype(NP16))
        in_maps.append({"wseq": wseq, "kib": kib, "ws": ws16, "s0": s0})
    return in_maps


def assemble_output(results, t_steps=T):
    ni = BPC * t_steps
    out = np.empty((B * t_steps, V), np.float32)
    for c in range(N_CORES):
        blk = results[c]["logits"]  # [ni, V] fp16, rows i = t*8+b
        out[c * ni:(c + 1) * ni] = (
            blk.reshape(t_steps, BPC, V).transpose(1, 0, 2)
            .reshape(ni, V).astype(np.float32))
    return out


_NC_CACHE = {}


def _get_nc(t_steps=T):
    if t_steps not in _NC_CACHE:
        _NC_CACHE[t_steps] = build_nc(t_steps)
    return _NC_CACHE[t_steps]


_WARMED = set()


def kernel(user_ids, item_ids, h0, gate_ku, gate_ki, gate_bias,
           cand_ku, cand_ki, cand_bias, ws, trace=False):
    nc = _get_nc(T)
    in_maps = prep_inputs(np.asarray(user_ids), np.asarray(item_ids),
                          np.asarray(h0), np.asarray(gate_ku),
                          np.asarray(gate_ki), np.asarray(gate_bias),
                          np.asarray(cand_ku), np.asarray(cand_ki),
                          np.asarray(cand_bias), np.asarray(ws))
    # First execution of a fresh NEFF can race its initial input DMAs against
    # the first compute instructions; run once untraced to warm the device.
    if id(nc) not in _WARMED:
        bass_utils.run_bass_kernel_spmd(
            nc, in_maps, core_ids=list(range(N_CORES)), trace=False)
        _WARMED.add(id(nc))
    res = bass_utils.run_bass_kernel_spmd(
        nc, in_maps, core_ids=list(range(N_CORES)), trace=trace)
    out = assemble_output(res.results)
    if trace:
        kernel.last_result = res
    return out


# revision 9
# speedup vs baseline: 1.2991x; 1.0277x over previous
"""Trainium2 Bass kernel for nn_CollaborativeRNNModel.

Model (per reference):
  per step t (T=100), batch b (B=64), hidden H=128:
    u   = sigmoid(h @ gate_ku[uid,:,128:] + gate_bias[128:] + gate_ki[iid,128:])
    c   = tanh(h @ cand_ku[uid] + cand_bias + cand_ki[iid])
    h'  = u*h + (1-u)*c
  logits = states[B*T, H] @ ws[H, 20001]

Sharding: data-parallel over batch, 8 rows per core.  Per-user weights are
pre-gathered host-side into a fp8(e3m4) stream (one 512KB DMA per 2 steps).

Device-side structure:
  - State is stored as S = 2h (fp16).  Both gate matmuls use rhs = S with
    host-side weight scales gamma_u=32, gamma_c=64, so PSUM holds 64*x_u and
    128*x_c.  A SINGLE tanh activation per step (scale 1/128) yields
    t_u = tanh(x_u/2) (u = (1+t_u)/2) and t_c = tanh(x_c) = c; biases and
    item embeddings are pre-folded into a fp32 PSUM preload (the preload for
    step t+1 is issued before the t update so it runs in the matmul window).
  - The state update S' = (S/2 + t_c) + t_u*(S/2 - t_c) takes 4 fused DVE
    ops; per-step chain is MMs -> tanh -> 4 DVE ops -> next MMs.
  - The logits matmul (lhsT = S columns, rhs = fp16 ws; logits are tiny so
    fp8 ws would alone cost ~1.3e-2 rel err) is interleaved in 512-col PSUM
    chunks whose matmuls are issued one step BEFORE their drains, so drains
    (scale 1/64, fp32 PSUM -> fp16 SBUF; 1 on vector + up to 2 on scalar per
    step) never stall an engine FIFO waiting on the PE.  2048-col staging
    tiles stream out on the gpsimd queue (tail across all three queues).
  - A warmup execution inside kernel() absorbs a first-run race between the
    initial input DMAs and the first compute instructions of a fresh NEFF.
"""

import numpy as np
import ml_dtypes  # noqa: F401  (np fp8 dtype support)

import concourse.bass as bass  # noqa: F401
import concourse.bacc as bacc
import concourse.tile as tile
import concourse.mybir as mybir
import concourse.bass_utils as bass_utils

H = 128
U = 5000
I = 20000
B = 64
T = 100
N_CORES = 8
BPC = B // N_CORES          # batch rows per core = 8
V = I + 1                   # vocab/items = 20001
NI = BPC * T                # rows per core = 800
CHUNK = 1024                # logits PSUM chunk (two banks)
STG = 2048                  # output staging width (2 chunks per unit)
PFW = 3                     # wseq prefetch depth (2-step chunks)
PFIN_BUFS = 3               # in-flight logits PSUM chunks
F32 = mybir.dt.float32
F16 = mybir.dt.float16
F8 = mybir.dt.float8e3
NP16 = np.float16

F8MAX = 15.5
G_U = 32.0                  # gate (u-half) weight scale
G_C = 64.0                  # cand weight scale
ACT_SCALE = 1.0 / 128.0     # tanh input scale
DRAIN_SCALE = 1.0           # drains are unscaled; host multiplies by 0.5


def build_nc(t_steps=T):
    """Build and compile the per-core Bass program (SPMD, same on all cores)."""
    ni = BPC * t_steps
    n_mtiles = (ni + 127) // 128
    n_wchunks = t_steps // 2

    nc = bacc.Bacc("TRN2", target_bir_lowering=False, debug=False,
                   enable_asserts=False, num_devices=N_CORES)

    # DRAM inputs (per core)
    # wseq[s*128 + k, st*2048 + b*256 + j] for step t = 2s+st:
    #   j<128: 32*gate_ku[uid(t,b), k, 128+j];  j>=128: 64*cand_ku[uid(t,b), k, j-128]
    wseq_d = nc.dram_tensor("wseq", [n_wchunks * H, 2 * BPC * 2 * H], F8,
                            kind="ExternalInput")
    # kib[k, t*16 + j] = 64*(gate_ki[iid(t,b=j),128+k] + gate_bias[128+k])   (j<8)
    #                    128*(cand_ki[iid(t,b=j-8),k] + cand_bias[k])        (j>=8)
    kib_d = nc.dram_tensor("kib", [H, 16 * t_steps], F32, kind="ExternalInput")
    ws_d = nc.dram_tensor("ws", [H, V], F16, kind="ExternalInput")
    s0_d = nc.dram_tensor("s0", [H, BPC], F16, kind="ExternalInput")
    out_d = nc.dram_tensor("logits", [ni, V], F16, kind="ExternalOutput")

    with tile.TileContext(nc) as tc:
        with (
            tc.tile_pool(name="big", bufs=1) as bpool,
            tc.tile_pool(name="w", bufs=PFW + 2) as wpool,
            tc.tile_pool(name="sm", bufs=4) as spool,
            tc.tile_pool(name="stage", bufs=4) as stpool,
            tc.tile_pool(name="prec", bufs=2, space="PSUM") as prec,
            tc.tile_pool(name="pfin", bufs=PFIN_BUFS, space="PSUM") as pfin,
        ):
            # ---- one-time loads ----
            kib = bpool.tile([128, 16 * t_steps], F32, tag="kib")
            nc.scalar.dma_start(kib[:], kib_d.ap())

            # statesT[k, t*8 + b] = S col b BEFORE step t (t=0 -> 2*h0)
            statesT = bpool.tile([H, BPC * (t_steps + 1)], F16, tag="statesT")
            nc.scalar.dma_start(statesT[:, 0:BPC], s0_d.ap())

            # weight-stream prefetch (2 steps per chunk)
            wt_tiles = {}

            def issue_wt(s):
                wt = wpool.tile([128, 2 * BPC * 2 * H], F8, tag="wt")
                nc.sync.dma_start(wt[:], wseq_d.ap()[s * H:(s + 1) * H, :])
                wt_tiles[s] = wt

            for s in range(min(PFW, n_wchunks)):
                issue_wt(s)

            # ws resident in SBUF (first needed at ~step 17)
            ws_sb = bpool.tile([H, V], F16, tag="ws")
            nc.scalar.dma_start(ws_sb[:], ws_d.ap())

            # ---- interleaved logits-matmul machinery ----
            # chunk lifecycle: mm_pending -> (chunk MMs issued, pfin tile
            # in flight) -> drain_ready -> (drained into stage) -> unit DMA
            mm_pending = []     # (ci, cw, unit_key)
            drain_ready = []    # (pf_tile, ci, cw, unit_key, t_issued)
            dve_halves = []     # (pf, ci, cw, key, chunk_rem)
            unit_state = {}     # key -> [stage, nrem, lo, mw, cg, gw]
            m_queued = [0]
            dma_rot = [0]

            def queue_mtile():
                m = m_queued[0]
                lo = m * 128
                mw = min(128, ni - lo)
                for cg in range(0, V, STG):
                    gw = min(STG, V - cg)
                    key = (m, cg)
                    unit_state[key] = [None, 0, lo, mw, cg, gw]
                    for ci in range(cg, cg + gw, CHUNK):
                        cw = min(CHUNK, cg + gw - ci)
                        mm_pending.append((ci, cw, key))
                        unit_state[key][1] += 1
                m_queued[0] += 1

            def issue_chunk_mms(maxn, t_now=10**9):
                # leave one pfin buffer as slack for drains still executing,
                # so a chunk matmul never blocks the PE queue on buffer reuse
                n = min(maxn, PFIN_BUFS - 1 - len(drain_ready),
                        len(mm_pending))
                for _ in range(max(0, n)):
                    ci, cw, key = mm_pending.pop(0)
                    lo, mw = unit_state[key][2], unit_state[key][3]
                    lhs = statesT[:, BPC + lo: BPC + lo + mw]
                    pf = pfin.tile([128, CHUNK], F32, tag="pf", name="pf")
                    for q in range(0, cw, 512):
                        qw = min(512, cw - q)
                        nc.tensor.matmul(pf[:mw, q:q + qw], lhsT=lhs,
                                         rhs=ws_sb[:, ci + q:ci + q + qw],
                                         start=True, stop=True,
                                         skip_group_check=True)
                    drain_ready.append((pf, ci, cw, key, t_now))

            def _chunk_done(key, tail):
                st = unit_state[key]
                st[1] -= 1
                if st[1] == 0:
                    _, _, lo, mw, cg, gw = st
                    if tail:
                        eng = (nc.gpsimd, nc.scalar, nc.sync)[dma_rot[0] % 3]
                        dma_rot[0] += 1
                    else:
                        eng = nc.gpsimd
                    eng.dma_start(out_d.ap()[lo:lo + mw, cg:cg + gw],
                                  st[0][:mw, :gw])
                    del unit_state[key]

            def _stage_for(key):
                st = unit_state[key]
                if st[0] is None:
                    st[0] = stpool.tile([128, STG], F16, tag="stg",
                                        name="stg")
                return st[0]

            def drain_chunk_scalar(t_now=10**9, tail=False):
                # only drain chunks whose matmuls are >= 1 step old, so the
                # drain is ready the moment the scalar engine finishes tanh
                if not drain_ready or drain_ready[0][4] >= t_now:
                    return
                pf, ci, cw, key, _ = drain_ready.pop(0)
                st = unit_state[key]
                mw, cg = st[3], st[4]
                stg = _stage_for(key)
                nc.scalar.activation(stg[:mw, ci - cg:ci - cg + cw],
                                     pf[:mw, :cw],
                                     mybir.ActivationFunctionType.Copy,
                                     bias=0.0, scale=1.0)
                _chunk_done(key, tail)

            def drain_chunk_vector_tail():
                if not drain_ready:
                    return
                pf, ci, cw, key, _ = drain_ready.pop(0)
                st = unit_state[key]
                mw, cg = st[3], st[4]
                stg = _stage_for(key)
                nc.vector.tensor_copy(stg[:mw, ci - cg:ci - cg + cw],
                                      pf[:mw, :cw])
                _chunk_done(key, tail=True)

            def assign_dve_chunk(t_now):
                if not drain_ready or drain_ready[0][4] >= t_now:
                    return
                pf, ci, cw, key, _ = drain_ready.pop(0)
                rem = [0]
                for off in range(0, cw, 512):
                    w = min(512, cw - off)
                    # (pf, local offset in pf tile, global col, width, ...)
                    dve_halves.append((pf, off, ci + off, w, key, rem))
                    rem[0] += 1

            def drain_half_vector(t):
                # the (statesT*0.0)+pf form injects a read dependency on the
                # state column written THIS step, so the vector engine only
                # starts the drain after the update chain - never inside it
                if not dve_halves:
                    return
                pf, off, gci, w, key, rem = dve_halves.pop(0)
                st = unit_state[key]
                mw, cg = st[3], st[4]
                stg = _stage_for(key)
                hi = (t + 2) * BPC
                win = statesT[:, hi - w:hi]
                nc.vector.scalar_tensor_tensor(
                    out=stg[:mw, gci - cg:gci - cg + w],
                    in0=win[:mw, :], scalar=0.0,
                    in1=pf[:mw, off:off + w],
                    op0=mybir.AluOpType.mult, op1=mybir.AluOpType.add)
                rem[0] -= 1
                if rem[0] == 0:
                    _chunk_done(key, tail=False)

            # ---- recurrence ----
            ps_tiles = {}
            ps_tiles[0] = prec.tile([128, 2 * BPC], F32, tag="ps", name="ps")
            nc.vector.tensor_copy(ps_tiles[0][:], kib[:, 0:16])

            for t in range(t_steps):
                if t % 2 == 0:
                    s = t // 2
                    if s + PFW < n_wchunks:
                        issue_wt(s + PFW)
                wt = wt_tiles[t // 2]
                if t % 2 == 1:
                    del wt_tiles[t // 2]
                wbase = (t % 2) * (BPC * 2 * H)
                ps = ps_tiles.pop(t)

                for b in range(BPC):
                    scol = statesT[:, t * BPC + b: t * BPC + b + 1]
                    w0 = wbase + b * 256
                    nc.tensor.matmul(ps[:, b:b + 1],
                                     lhsT=wt[:, w0:w0 + 128],
                                     rhs=scol, start=False, stop=True,
                                     skip_group_check=True)
                    nc.tensor.matmul(ps[:, BPC + b:BPC + b + 1],
                                     lhsT=wt[:, w0 + 128:w0 + 256],
                                     rhs=scol, start=False, stop=True,
                                     skip_group_check=True)

                # logits chunk MMs ride the PE queue right behind the small
                # MMs; their drains happen NEXT step so they never stall.
                while m_queued[0] < n_mtiles and \
                        (m_queued[0] + 1) * 128 <= (t + 1) * BPC:
                    queue_mtile()
                issue_chunk_mms(3, t_now=t)

                # preload for t+1 BEFORE the update ops: runs on DVE during
                # the matmul/tanh window, off the critical path.
                if t + 1 < t_steps:
                    psn = prec.tile([128, 2 * BPC], F32, tag="ps", name="ps")
                    nc.vector.tensor_copy(psn[:],
                                          kib[:, (t + 1) * 16:(t + 2) * 16])
                    ps_tiles[t + 1] = psn

                # one tanh for both halves: t_u = tanh(x_u/2), t_c = tanh(x_c)
                tt = spool.tile([128, 2 * BPC], F16, tag="tt")
                nc.scalar.activation(tt[:], ps[:],
                                     mybir.ActivationFunctionType.Tanh,
                                     scale=ACT_SCALE)

                # S' = (S/2 + t_c) + t_u*(S/2 - t_c)
                scur = statesT[:, t * BPC:(t + 1) * BPC]
                t_u = tt[:, 0:BPC]
                t_c = tt[:, BPC:2 * BPC]
                t0 = spool.tile([128, BPC], F16, tag="t0")
                nc.vector.scalar_tensor_tensor(
                    out=t0[:], in0=scur, scalar=0.5, in1=t_c,
                    op0=mybir.AluOpType.mult, op1=mybir.AluOpType.add)
                t1 = spool.tile([128, BPC], F16, tag="t1")
                nc.vector.scalar_tensor_tensor(
                    out=t1[:], in0=scur, scalar=0.5, in1=t_c,
                    op0=mybir.AluOpType.mult, op1=mybir.AluOpType.subtract)
                t2 = spool.tile([128, BPC], F16, tag="t2")
                nc.vector.tensor_mul(t2[:], t_u, t1[:])
                nc.vector.tensor_add(statesT[:, (t + 1) * BPC:(t + 2) * BPC],
                                     t0[:], t2[:])

                # drains for chunks whose MMs were issued in prior steps
                drain_chunk_scalar(t_now=t)
                if t >= 64 and t % 2 == 1:
                    assign_dve_chunk(t_now=t)
                if t >= 64:
                    drain_half_vector(t)

            # ---- tail ----
            while m_queued[0] < n_mtiles:
                queue_mtile()
            while dve_halves:
                drain_half_vector(t_steps - 2)
            use_scalar = True
            while mm_pending or drain_ready:
                issue_chunk_mms(2)
                if use_scalar:
                    drain_chunk_scalar(tail=True)
                else:
                    drain_chunk_vector_tail()
                use_scalar = not use_scalar

    nc.compile()
    return nc


def prep_inputs(user_ids, item_ids, h0, gate_ku, gate_ki, gate_bias,
                cand_ku, cand_ki, cand_bias, ws, t_steps=T):
    """Host-side sharding/layout -> per-core in_maps."""
    f8 = ml_dtypes.float8_e3m4
    wu8 = np.clip(np.ascontiguousarray(gate_ku[:, :, H:]) * G_U,
                  -F8MAX, F8MAX).astype(f8)
    wc8 = np.clip(np.ascontiguousarray(cand_ku) * G_C,
                  -F8MAX, F8MAX).astype(f8)
    ws16 = np.ascontiguousarray(ws).astype(NP16)
    # folded ki + bias tables (fp32)
    gki = (np.asarray(gate_ki[:, H:], np.float32)
           + np.asarray(gate_bias[H:], np.float32)) * 64.0      # [V, 128]
    cki = (np.asarray(cand_ki, np.float32)
           + np.asarray(cand_bias, np.float32)) * 128.0          # [V, 128]

    in_maps = []
    for c in range(N_CORES):
        rows = slice(c * BPC, (c + 1) * BPC)
        uid_t = np.asarray(user_ids[rows, :t_steps], np.int64).T    # [T, 8]
        iid_t = np.asarray(item_ids[rows, :t_steps], np.int64).T    # [T, 8]
        # weight stream: [T,8,128,256] -> [T,128,8,256] -> 2-step chunks
        wseq = np.concatenate([wu8[uid_t], wc8[uid_t]], axis=3)
        wseq = np.ascontiguousarray(wseq.transpose(0, 2, 1, 3)).reshape(
            t_steps * H, BPC * 2 * H)
        wseq = np.ascontiguousarray(
            wseq.reshape(t_steps // 2, 2, H, BPC * 2 * H)
            .transpose(0, 2, 1, 3)).reshape(t_steps // 2 * H, 2 * BPC * 2 * H)
        # kib columns per step: [u(8) | c(8)], transposed to [128, 16T]
        kib = np.concatenate([gki[iid_t], cki[iid_t]], axis=1)   # [T,16,128]
        kib = np.ascontiguousarray(kib.reshape(t_steps * 16, H).T,
                                   np.float32)
        s0 = np.ascontiguousarray((2.0 * np.asarray(h0[rows], np.float32)).T
                                  .astype(NP16))
        in_maps.append({"wseq": wseq, "kib": kib, "ws": ws16, "s0": s0})
    return in_maps


def assemble_output(results, t_steps=T):
    ni = BPC * t_steps
    out = np.empty((B * t_steps, V), np.float32)
    for c in range(N_CORES):
        blk = results[c]["logits"]  # [ni, V] fp16, rows i = t*8+b
        out[c * ni:(c + 1) * ni] = (
            blk.reshape(t_steps, BPC, V).transpose(1, 0, 2)
            .reshape(ni, V).astype(np.float32)) * 0.5
    return out


_NC_CACHE = {}


def _get_nc(t_steps=T):
    if t_steps not in _NC_CACHE:
        _NC_CACHE[t_steps] = build_nc(t_steps)
    return _NC_CACHE[t_steps]


_WARMED = set()


def kernel(user_ids, item_ids, h0, gate_ku, gate_ki, gate_bias,
           cand_ku, cand_ki, cand_bias, ws, trace=False):
    nc = _get_nc(T)
    in_maps = prep_inputs(np.asarray(user_ids), np.asarray(item_ids),
                          np.asarray(h0), np.asarray(gate_ku),
                          np.asarray(gate_ki), np.asarray(gate_bias),
                          np.asarray(cand_ku), np.asarray(cand_ki),
                          np.asarray(cand_bias), np.asarray(ws))
    # First execution of a fresh NEFF can race its initial input DMAs against
    # the first compute instructions; run once untraced to warm the device.
    if id(nc) not in _WARMED:
        bass_utils.run_bass_kernel_spmd(
            nc, in_maps, core_ids=list(range(N_CORES)), trace=False)
        _WARMED.add(id(nc))
    res = bass_utils.run_bass_kernel_spmd(
        nc, in_maps, core_ids=list(range(N_CORES)), trace=trace)
    out = assemble_output(res.results)
    if trace:
        kernel.last_result = res
    return out


# revision 10
# speedup vs baseline: 1.3288x; 1.0229x over previous
"""Trainium2 Bass kernel for nn_CollaborativeRNNModel.

Model (per reference):
  per step t (T=100), batch b (B=64), hidden H=128:
    u   = sigmoid(h @ gate_ku[uid,:,128:] + gate_bias[128:] + gate_ki[iid,128:])
    c   = tanh(h @ cand_ku[uid] + cand_bias + cand_ki[iid])
    h'  = u*h + (1-u)*c
  logits = states[B*T, H] @ ws[H, 20001]

Sharding: data-parallel over batch, 8 rows per core.  Per-user weights are
pre-gathered host-side into a fp8(e3m4) stream (one 512KB DMA per 2 steps).

Device-side structure:
  - State is stored as S = 2h (fp16).  Both gate matmuls use rhs = S with
    host-side weight scales gamma_u=32, gamma_c=64, so PSUM holds 64*x_u and
    128*x_c.  A SINGLE tanh activation per step (scale 1/128) yields
    t_u = tanh(x_u/2) (u = (1+t_u)/2) and t_c = tanh(x_c) = c; biases and
    item embeddings are pre-folded into a fp32 PSUM preload (the preload for
    step t+1 is issued before the t update so it runs in the matmul window).
  - The state update S' = (S/2 + t_c) + t_u*(S/2 - t_c) takes 4 fused DVE
    ops; per-step chain is MMs -> tanh -> 4 DVE ops -> next MMs.
  - The logits matmul (lhsT = S columns, rhs = fp16 ws; logits are tiny so
    fp8 ws would alone cost ~1.3e-2 rel err) is interleaved in 512-col PSUM
    chunks whose matmuls are issued one step BEFORE their drains, so drains
    (scale 1/64, fp32 PSUM -> fp16 SBUF; 1 on vector + up to 2 on scalar per
    step) never stall an engine FIFO waiting on the PE.  2048-col staging
    tiles stream out on the gpsimd queue (tail across all three queues).
  - A warmup execution inside kernel() absorbs a first-run race between the
    initial input DMAs and the first compute instructions of a fresh NEFF.
"""

import numpy as np
import ml_dtypes  # noqa: F401  (np fp8 dtype support)

import concourse.bass as bass  # noqa: F401
import concourse.bacc as bacc
import concourse.tile as tile
import concourse.mybir as mybir
import concourse.bass_utils as bass_utils

H = 128
U = 5000
I = 20000
B = 64
T = 100
N_CORES = 8
BPC = B // N_CORES          # batch rows per core = 8
V = I + 1                   # vocab/items = 20001
NI = BPC * T                # rows per core = 800
CHUNK = 1024                # logits PSUM chunk (two banks)
STG = 2048                  # output staging width (2 chunks per unit)
PFW = 3                     # wseq prefetch depth (2-step chunks)
PFIN_BUFS = 3               # in-flight logits PSUM chunks
F32 = mybir.dt.float32
F16 = mybir.dt.float16
F8 = mybir.dt.float8e3
NP16 = np.float16

F8MAX = 15.5
G_U = 32.0                  # gate (u-half) weight scale
G_C = 64.0                  # cand weight scale
ACT_SCALE = 1.0 / 128.0     # tanh input scale
DRAIN_SCALE = 1.0           # drains are unscaled; host multiplies by 0.5


def build_nc(t_steps=T):
    """Build and compile the per-core Bass program (SPMD, same on all cores)."""
    ni = BPC * t_steps
    n_mtiles = (ni + 127) // 128
    n_wchunks = t_steps // 2

    nc = bacc.Bacc("TRN2", target_bir_lowering=False, debug=False,
                   enable_asserts=False, num_devices=N_CORES)

    # DRAM inputs (per core)
    # wseq[s*128 + k, st*2048 + b*256 + j] for step t = 2s+st:
    #   j<128: 32*gate_ku[uid(t,b), k, 128+j];  j>=128: 64*cand_ku[uid(t,b), k, j-128]
    wseq_d = nc.dram_tensor("wseq", [n_wchunks * H, 2 * BPC * 2 * H], F8,
                            kind="ExternalInput")
    # kib[k, t*16 + j] = 64*(gate_ki[iid(t,b=j),128+k] + gate_bias[128+k])   (j<8)
    #                    128*(cand_ki[iid(t,b=j-8),k] + cand_bias[k])        (j>=8)
    kib_d = nc.dram_tensor("kib", [H, 16 * t_steps], F32, kind="ExternalInput")
    ws_d = nc.dram_tensor("ws", [H, V], F16, kind="ExternalInput")
    s0_d = nc.dram_tensor("s0", [H, BPC], F16, kind="ExternalInput")
    out_d = nc.dram_tensor("logits", [ni, V], F16, kind="ExternalOutput")

    with tile.TileContext(nc) as tc:
        with (
            tc.tile_pool(name="big", bufs=1) as bpool,
            tc.tile_pool(name="w", bufs=PFW + 2) as wpool,
            tc.tile_pool(name="sm", bufs=4) as spool,
            tc.tile_pool(name="stage", bufs=4) as stpool,
            tc.tile_pool(name="prec", bufs=2, space="PSUM") as prec,
            tc.tile_pool(name="pfin", bufs=PFIN_BUFS, space="PSUM") as pfin,
        ):
            # ---- one-time loads ----
            kib = bpool.tile([128, 16 * t_steps], F32, tag="kib")
            nc.scalar.dma_start(kib[:], kib_d.ap())

            # statesT[k, t*8 + b] = S col b BEFORE step t (t=0 -> 2*h0)
            statesT = bpool.tile([H, BPC * (t_steps + 1)], F16, tag="statesT")
            nc.scalar.dma_start(statesT[:, 0:BPC], s0_d.ap())

            # weight-stream prefetch (2 steps per chunk)
            wt_tiles = {}

            def issue_wt(s):
                wt = wpool.tile([128, 2 * BPC * 2 * H], F8, tag="wt")
                nc.sync.dma_start(wt[:], wseq_d.ap()[s * H:(s + 1) * H, :])
                wt_tiles[s] = wt

            for s in range(min(PFW, n_wchunks)):
                issue_wt(s)

            # ws resident in SBUF (first needed at ~step 17)
            ws_sb = bpool.tile([H, V], F16, tag="ws")
            nc.gpsimd.dma_start(ws_sb[:], ws_d.ap())

            # ---- interleaved logits-matmul machinery ----
            # chunk lifecycle: mm_pending -> (chunk MMs issued, pfin tile
            # in flight) -> drain_ready -> (drained into stage) -> unit DMA
            mm_pending = []     # (ci, cw, unit_key)
            drain_ready = []    # (pf_tile, ci, cw, unit_key, t_issued)
            dve_halves = []     # (pf, ci, cw, key, chunk_rem)
            unit_state = {}     # key -> [stage, nrem, lo, mw, cg, gw]
            m_queued = [0]
            dma_rot = [0]

            def queue_mtile():
                m = m_queued[0]
                lo = m * 128
                mw = min(128, ni - lo)
                for cg in range(0, V, STG):
                    gw = min(STG, V - cg)
                    key = (m, cg)
                    unit_state[key] = [None, 0, lo, mw, cg, gw]
                    for ci in range(cg, cg + gw, CHUNK):
                        cw = min(CHUNK, cg + gw - ci)
                        mm_pending.append((ci, cw, key))
                        unit_state[key][1] += 1
                m_queued[0] += 1

            def issue_chunk_mms(maxn, t_now=10**9):
                # leave one pfin buffer as slack for drains still executing,
                # so a chunk matmul never blocks the PE queue on buffer reuse
                n = min(maxn, PFIN_BUFS - 1 - len(drain_ready),
                        len(mm_pending))
                for _ in range(max(0, n)):
                    ci, cw, key = mm_pending.pop(0)
                    lo, mw = unit_state[key][2], unit_state[key][3]
                    lhs = statesT[:, BPC + lo: BPC + lo + mw]
                    pf = pfin.tile([128, CHUNK], F32, tag="pf", name="pf")
                    for q in range(0, cw, 512):
                        qw = min(512, cw - q)
                        nc.tensor.matmul(pf[:mw, q:q + qw], lhsT=lhs,
                                         rhs=ws_sb[:, ci + q:ci + q + qw],
                                         start=True, stop=True,
                                         skip_group_check=True)
                    drain_ready.append((pf, ci, cw, key, t_now))

            def _chunk_done(key, tail):
                st = unit_state[key]
                st[1] -= 1
                if st[1] == 0:
                    _, _, lo, mw, cg, gw = st
                    if tail:
                        eng = (nc.gpsimd, nc.scalar, nc.sync)[dma_rot[0] % 3]
                        dma_rot[0] += 1
                    else:
                        eng = nc.gpsimd
                    eng.dma_start(out_d.ap()[lo:lo + mw, cg:cg + gw],
                                  st[0][:mw, :gw])
                    del unit_state[key]

            def _stage_for(key):
                st = unit_state[key]
                if st[0] is None:
                    st[0] = stpool.tile([128, STG], F16, tag="stg",
                                        name="stg")
                return st[0]

            def drain_chunk_scalar(t_now=10**9, tail=False):
                # only drain chunks whose matmuls are >= 1 step old, so the
                # drain is ready the moment the scalar engine finishes tanh
                if not drain_ready or drain_ready[0][4] >= t_now:
                    return
                pf, ci, cw, key, _ = drain_ready.pop(0)
                st = unit_state[key]
                mw, cg = st[3], st[4]
                stg = _stage_for(key)
                nc.scalar.activation(stg[:mw, ci - cg:ci - cg + cw],
                                     pf[:mw, :cw],
                                     mybir.ActivationFunctionType.Copy,
                                     bias=0.0, scale=1.0)
                _chunk_done(key, tail)

            def drain_chunk_vector_tail():
                if not drain_ready:
                    return
                pf, ci, cw, key, _ = drain_ready.pop(0)
                st = unit_state[key]
                mw, cg = st[3], st[4]
                stg = _stage_for(key)
                nc.vector.tensor_copy(stg[:mw, ci - cg:ci - cg + cw],
                                      pf[:mw, :cw])
                _chunk_done(key, tail=True)

            def assign_dve_chunk(t_now):
                if not drain_ready or drain_ready[0][4] >= t_now:
                    return
                pf, ci, cw, key, _ = drain_ready.pop(0)
                rem = [0]
                for off in range(0, cw, 512):
                    w = min(512, cw - off)
                    # (pf, local offset in pf tile, global col, width, ...)
                    dve_halves.append((pf, off, ci + off, w, key, rem))
                    rem[0] += 1

            def drain_half_vector(t):
                # the (statesT*0.0)+pf form injects a read dependency on the
                # state column written THIS step, so the vector engine only
                # starts the drain after the update chain - never inside it
                if not dve_halves:
                    return
                pf, off, gci, w, key, rem = dve_halves.pop(0)
                st = unit_state[key]
                mw, cg = st[3], st[4]
                stg = _stage_for(key)
                hi = (t + 2) * BPC
                win = statesT[:, hi - w:hi]
                nc.vector.scalar_tensor_tensor(
                    out=stg[:mw, gci - cg:gci - cg + w],
                    in0=win[:mw, :], scalar=0.0,
                    in1=pf[:mw, off:off + w],
                    op0=mybir.AluOpType.mult, op1=mybir.AluOpType.add)
                rem[0] -= 1
                if rem[0] == 0:
                    _chunk_done(key, tail=False)

            # ---- recurrence ----
            ps_tiles = {}
            ps_tiles[0] = prec.tile([128, 2 * BPC], F32, tag="ps", name="ps")
            nc.vector.tensor_copy(ps_tiles[0][:], kib[:, 0:16])

            for t in range(t_steps):
                if t % 2 == 0:
                    s = t // 2
                    if s + PFW < n_wchunks:
                        issue_wt(s + PFW)
                wt = wt_tiles[t // 2]
                if t % 2 == 1:
                    del wt_tiles[t // 2]
                wbase = (t % 2) * (BPC * 2 * H)
                ps = ps_tiles.pop(t)

                for b in range(BPC):
                    scol = statesT[:, t * BPC + b: t * BPC + b + 1]
                    w0 = wbase + b * 256
                    nc.tensor.matmul(ps[:, b:b + 1],
                                     lhsT=wt[:, w0:w0 + 128],
                                     rhs=scol, start=False, stop=True,
                                     skip_group_check=True)
                    nc.tensor.matmul(ps[:, BPC + b:BPC + b + 1],
                                     lhsT=wt[:, w0 + 128:w0 + 256],
                                     rhs=scol, start=False, stop=True,
                                     skip_group_check=True)

                # logits chunk MMs ride the PE queue right behind the small
                # MMs; their drains happen NEXT step so they never stall.
                while m_queued[0] < n_mtiles and \
                        (m_queued[0] + 1) * 128 <= (t + 1) * BPC:
                    queue_mtile()
                issue_chunk_mms(3, t_now=t)

                # preload for t+1 BEFORE the update ops: runs on DVE during
                # the matmul/tanh window, off the critical path.
                if t + 1 < t_steps:
                    psn = prec.tile([128, 2 * BPC], F32, tag="ps", name="ps")
                    nc.vector.tensor_copy(psn[:],
                                          kib[:, (t + 1) * 16:(t + 2) * 16])
                    ps_tiles[t + 1] = psn

                # one tanh for both halves: t_u = tanh(x_u/2), t_c = tanh(x_c)
                tt = spool.tile([128, 2 * BPC], F16, tag="tt")
                nc.scalar.activation(tt[:], ps[:],
                                     mybir.ActivationFunctionType.Tanh,
                                     scale=ACT_SCALE)

                # S' = (S/2 + t_c) + t_u*(S/2 - t_c)
                scur = statesT[:, t * BPC:(t + 1) * BPC]
                t_u = tt[:, 0:BPC]
                t_c = tt[:, BPC:2 * BPC]
                t0 = spool.tile([128, BPC], F16, tag="t0")
                nc.vector.scalar_tensor_tensor(
                    out=t0[:], in0=scur, scalar=0.5, in1=t_c,
                    op0=mybir.AluOpType.mult, op1=mybir.AluOpType.add)
                t1 = spool.tile([128, BPC], F16, tag="t1")
                nc.vector.scalar_tensor_tensor(
                    out=t1[:], in0=scur, scalar=0.5, in1=t_c,
                    op0=mybir.AluOpType.mult, op1=mybir.AluOpType.subtract)
                t2 = spool.tile([128, BPC], F16, tag="t2")
                nc.vector.tensor_mul(t2[:], t_u, t1[:])
                nc.vector.tensor_add(statesT[:, (t + 1) * BPC:(t + 2) * BPC],
                                     t0[:], t2[:])

                # drains for chunks whose MMs were issued in prior steps
                drain_chunk_scalar(t_now=t)
                if t >= 64 and t % 2 == 1:
                    assign_dve_chunk(t_now=t)
                if t >= 64:
                    drain_half_vector(t)

            # ---- tail ----
            while m_queued[0] < n_mtiles:
                queue_mtile()
            while dve_halves:
                drain_half_vector(t_steps - 2)
            while mm_pending or drain_ready:
                issue_chunk_mms(2, t_now=0)
                drain_chunk_scalar(tail=True)
                drain_chunk_vector_tail()

    nc.compile()
    return nc


def prep_inputs(user_ids, item_ids, h0, gate_ku, gate_ki, gate_bias,
                cand_ku, cand_ki, cand_bias, ws, t_steps=T):
    """Host-side sharding/layout -> per-core in_maps."""
    f8 = ml_dtypes.float8_e3m4
    wu8 = np.clip(np.ascontiguousarray(gate_ku[:, :, H:]) * G_U,
                  -F8MAX, F8MAX).astype(f8)
    wc8 = np.clip(np.ascontiguousarray(cand_ku) * G_C,
                  -F8MAX, F8MAX).astype(f8)
    ws16 = np.ascontiguousarray(ws).astype(NP16)
    # folded ki + bias tables (fp32)
    gki = (np.asarray(gate_ki[:, H:], np.float32)
           + np.asarray(gate_bias[H:], np.float32)) * 64.0      # [V, 128]
    cki = (np.asarray(cand_ki, np.float32)
           + np.asarray(cand_bias, np.float32)) * 128.0          # [V, 128]

    in_maps = []
    for c in range(N_CORES):
        rows = slice(c * BPC, (c + 1) * BPC)
        uid_t = np.asarray(user_ids[rows, :t_steps], np.int64).T    # [T, 8]
        iid_t = np.asarray(item_ids[rows, :t_steps], np.int64).T    # [T, 8]
        # weight stream: [T,8,128,256] -> [T,128,8,256] -> 2-step chunks
        wseq = np.concatenate([wu8[uid_t], wc8[uid_t]], axis=3)
        wseq = np.ascontiguousarray(wseq.transpose(0, 2, 1, 3)).reshape(
            t_steps * H, BPC * 2 * H)
        wseq = np.ascontiguousarray(
            wseq.reshape(t_steps // 2, 2, H, BPC * 2 * H)
            .transpose(0, 2, 1, 3)).reshape(t_steps // 2 * H, 2 * BPC * 2 * H)
        # kib columns per step: [u(8) | c(8)], transposed to [128, 16T]
        kib = np.concatenate([gki[iid_t], cki[iid_t]], axis=1)   # [T,16,128]
        kib = np.ascontiguousarray(kib.reshape(t_steps * 16, H).T,
                                   np.float32)
        s0 = np.ascontiguousarray((2.0 * np.asarray(h0[rows], np.float32)).T
                                  .astype(NP16))
        in_maps.append({"wseq": wseq, "kib": kib, "ws": ws16, "s0": s0})
    return in_maps


def assemble_output(results, t_steps=T):
    ni = BPC * t_steps
    out = np.empty((B * t_steps, V), np.float32)
    for c in range(N_CORES):
        blk = results[c]["logits"]  # [ni, V] fp16, rows i = t*8+b
        out[c * ni:(c + 1) * ni] = (
            blk.reshape(t_steps, BPC, V).transpose(1, 0, 2)
            .reshape(ni, V).astype(np.float32)) * 0.5
    return out


_NC_CACHE = {}


def _get_nc(t_steps=T):
    if t_steps not in _NC_CACHE:
        _NC_CACHE[t_steps] = build_nc(t_steps)
    return _NC_CACHE[t_steps]


_WARMED = set()


def kernel(user_ids, item_ids, h0, gate_ku, gate_ki, gate_bias,
           cand_ku, cand_ki, cand_bias, ws, trace=False):
    nc = _get_nc(T)
    in_maps = prep_inputs(np.asarray(user_ids), np.asarray(item_ids),
                          np.asarray(h0), np.asarray(gate_ku),
                          np.asarray(gate_ki), np.asarray(gate_bias),
                          np.asarray(cand_ku), np.asarray(cand_ki),
                          np.asarray(cand_bias), np.asarray(ws))
    # First execution of a fresh NEFF can race its initial input DMAs against
    # the first compute instructions; run once untraced to warm the device.
    if id(nc) not in _WARMED:
        bass_utils.run_bass_kernel_spmd(
            nc, in_maps, core_ids=list(range(N_CORES)), trace=False)
        _WARMED.add(id(nc))
    res = bass_utils.run_bass_kernel_spmd(
        nc, in_maps, core_ids=list(range(N_CORES)), trace=trace)
    out = assemble_output(res.results)
    if trace:
        kernel.last_result = res
    return out
